# revision 17
# baseline (speedup 1.0000x reference)
"""Trainium2 Bass kernel for pairwise-channel-correlation pooling.

reference math (per sample, f: [256 ch, 25 pos]):
    G[i,j]  = sum_p (|f_ip + f_jp| - |f_ip - f_jp|)/2 * exp(T)
            = exp(T) * sum_p sign(f_ip) sign(f_jp) min(|f_ip|, |f_jp|)
    out     = G - rowmean(G) - colmean(G), then triu(row-major).

Device strategy (per core, 8 samples, pure data parallelism):
  - tiles of [(5 i-chans x 25 pos) = 125 partitions, 256 j] where
        t[q=(il,p), j] = clip(f_jp, -|f_ip|, +|f_ip|)
                       = sign(f_jp) * min(|f_ip|,|f_jp|)
    is ONE dual-op tensor_scalar (max, min) on the vector engine.
  - a stationary +-1 matrix S_g = sign(a)*mask_g on the tensor engine
    applies sign(f_ip), reduces over p, scatters the 5 channel rows into
    the PSUM block, and accumulates a column-sum row (exact: weights are
    only 0/+-1, accumulation fp32 in PSUM; fp32r streams 1 col/cycle).
  - row/col means + exp(T) scale applied from PSUM; host gathers triu.
"""

import sys

sys.path.insert(0, "/opt/trn_rl_repo")

import numpy as np

import concourse.bass as bass
import concourse.tile as tile
from concourse import bacc, mybir
from concourse import bass_utils

F32 = mybir.dt.float32
F32R = mybir.dt.float32r

B, D, H, W = 64, 256, 5, 5
HW = H * W  # 25
NCORES = 8
SPC = B // NCORES  # samples per core
NROW = 125  # K rows per tile: 5 chans x 25 pos
NT = 52  # tiles per sample (51 full + 1 tail)
NV = 25  # distinct mask variants
CS_ROW = 126  # psum row accumulating column sums

_BUILT = {}
_BUILT_LOOP = {}


def _build_masks() -> np.ndarray:
    """[125, NV*128] f32; variant v maps local row (il,p) -> col 5v+il,
    plus col CS_ROW=1 (column-sum accumulator row)."""
    m = np.zeros((NROW, NV, 128), dtype=np.float32)
    for v in range(NV):
        for il in range(5):
            m[il * 25 : (il + 1) * 25, v, 5 * v + il] = 1.0
    m[:, :, CS_ROW] = 1.0
    return np.ascontiguousarray(m.reshape(NROW, NV * 128))


def _build_kernel(loop_k=None):
    cache = _BUILT if loop_k is None else _BUILT_LOOP
    if "nc" in cache and cache.get("loop_k") == loop_k:
        return cache
    Alu = mybir.AluOpType
    Act = mybir.ActivationFunctionType

    nc = bacc.Bacc("TRN2", target_bir_lowering=False, debug=False,
                   num_devices=NCORES)
    b2_d = nc.dram_tensor("b2", [SPC, NROW, D], F32, kind="ExternalInput")
    a_d = nc.dram_tensor("a", [SPC, NROW, NT], F32, kind="ExternalInput")
    temp_d = nc.dram_tensor("temp", [1, 1], F32, kind="ExternalInput")
    out_d = nc.dram_tensor("out", [SPC, D, D], F32, kind="ExternalOutput")
    masks_d = nc.inline_tensor(_build_masks(), name="masks")
    ident_d = nc.inline_tensor(np.eye(NROW, dtype=np.float32), name="ident")

    with tile.TileContext(nc) as tc:
        with (
            tc.tile_pool(name="const", bufs=1) as cpool,
            tc.tile_pool(name="inp", bufs=2) as ipool,
            tc.tile_pool(name="acol", bufs=2) as apool,
            tc.tile_pool(name="sg", bufs=4) as spool,
            tc.tile_pool(name="tg", bufs=4) as tpool,
            tc.tile_pool(name="post", bufs=3) as opool,
            tc.tile_pool(name="psum", bufs=1, space="PSUM") as pspool,
        ):
            # ---- constants ----
            mask_sb = cpool.tile([NROW, NV * 128], F32)
            nc.sync.dma_start(mask_sb[:], masks_d.ap())
            ident = cpool.tile([NROW, NROW], F32)
            nc.sync.dma_start(ident[:], ident_d.ap())
            tsb = cpool.tile([1, 1], F32)
            nc.sync.dma_start(tsb[:], temp_d.ap())
            expT = cpool.tile([1, 1], F32)
            nc.scalar.activation(expT[:], tsb[:], Act.Exp)
            expT_col = cpool.tile([128, 1], F32)
            nc.gpsimd.partition_broadcast(expT_col[:], expT[:])

            import contextlib

            loop_cm = (
                tc.For_i(0, loop_k, 1) if loop_k is not None
                else contextlib.nullcontext()
            )
            with loop_cm:
                _emit_body(nc, tc, mask_sb, ident, expT, expT_col,
                           b2_d, a_d, out_d,
                           ipool, apool, spool, tpool, opool, pspool)

    nc.compile()
    cache.clear()
    cache["nc"] = nc
    cache["loop_k"] = loop_k
    return cache


def _emit_body(nc, tc, mask_sb, ident, expT, expT_col, b2_d, a_d, out_d,
               ipool, apool, spool, tpool, opool, pspool):
    Alu = mybir.AluOpType
    Act = mybir.ActivationFunctionType
    for s in range(SPC):
        b2 = ipool.tile([NROW, D], F32)
        nc.sync.dma_start(b2[:], b2_d.ap()[s])
        a = ipool.tile([NROW, NT], F32)
        nc.sync.dma_start(a[:], a_d.ap()[s])

        absA = apool.tile([NROW, NT], F32)
        nc.scalar.activation(absA[:], a[:], Act.Abs)
        negA = apool.tile([NROW, NT], F32)
        nc.vector.tensor_scalar_mul(negA[:], absA[:], -1.0)
        signA = apool.tile([NROW, NT], F32)
        nc.scalar.sign(signA[:], a[:])

        psums = [
            pspool.tile([128, D], F32, tag="ps", name=f"ps_{s}_{b}", bufs=6)
            for b in range(3)
        ]

        for g in range(NT):
            blk = g // 25 if g < 50 else 2
            v = g % 25
            first = (g % 25 == 0) if g < 50 else (g == 50)
            last = (g % 25 == 24) if g < 50 else (g == 51)

            Sg = spool.tile([NROW, 128], F32R)
            nc.gpsimd.tensor_scalar(
                Sg[:], mask_sb[:, v * 128 : (v + 1) * 128],
                signA[:, g : g + 1], None, Alu.mult,
            )
            tg = tpool.tile([NROW, D], F32R)
            nc.vector.tensor_scalar(
                tg[:], b2[:],
                negA[:, g : g + 1], absA[:, g : g + 1],
                Alu.max, Alu.min,
            )
            nc.tensor.matmul(
                psums[blk][:, :], Sg[:], tg[:], start=first, stop=last,
            )

        # psum -> sbuf with expT scale on ACT; fused row-sum accumulators
        gexp = [
            opool.tile([NROW, D], F32, tag="gexp", name=f"gexp_{s}_{b}", bufs=6)
            for b in range(3)
        ]
        rs = opool.tile([NROW, 4], F32, tag="rs")
        for blk in range(3):
            rows = NROW if blk < 2 else 6
            nc.scalar.activation(
                gexp[blk][:rows], psums[blk][:rows, :], Act.Copy,
                scale=expT_col[:rows], accum_out=rs[:rows, blk : blk + 1],
            )

        # column sums via symmetry: transpose the three row-sum segments
        # into one contiguous [1, 256] psum row.
        ps_t = pspool.tile([1, 2 * D], F32, tag="pst", bufs=2)
        nc.tensor.transpose(ps_t[:, 0:NROW], rs[:, 0:1], ident[:])
        nc.tensor.transpose(ps_t[:, NROW : 2 * NROW], rs[:, 1:2], ident[:])
        nc.tensor.transpose(ps_t[:, 2 * NROW : 2 * NROW + 6], rs[:6, 2:3], ident[:6, :6])
        cs_s = opool.tile([1, D], F32, tag="cs")
        nc.vector.tensor_scalar_mul(cs_s[:], ps_t[:, 0:D], 1.0 / D)
        cs_bc = opool.tile([128, D], F32, tag="csb")
        nc.gpsimd.partition_broadcast(cs_bc[:], cs_s[:])

        row0 = 0
        for blk in range(3):
            rows = NROW if blk < 2 else 6
            rmean = opool.tile([NROW, 1], F32, tag="rm", name=f"rm_{s}_{blk}")
            nc.vector.tensor_scalar_mul(
                rmean[:rows], rs[:rows, blk : blk + 1], 1.0 / D,
            )
            o2 = opool.tile([NROW, D], F32, tag="o2")
            nc.vector.scalar_tensor_tensor(
                o2[:rows], gexp[blk][:rows], rmean[:rows], cs_bc[:rows],
                Alu.subtract, Alu.subtract,
            )
            nc.sync.dma_start(out_d.ap()[s, row0 : row0 + rows, :], o2[:rows])
            row0 += rows


def _prep_core_inputs(f_core: np.ndarray, temp: np.ndarray) -> dict:
    """f_core: [SPC, D, H, W] -> relayout for the device kernel."""
    fr = f_core.reshape(SPC, D, HW).astype(np.float32)
    # b2[s] = f^T tiled 5x along partitions: [(il,p), j] = f[j, p]
    ft = np.transpose(fr, (0, 2, 1))  # [SPC, 25, 256]
    b2 = np.tile(ft, (1, 5, 1))  # [SPC, 125, 256]
    # a[s]: col g = f.flat[125g:125(g+1)], zero-padded to 52*125
    flat = fr.reshape(SPC, D * HW)
    pad = np.zeros((SPC, NT * NROW), dtype=np.float32)
    pad[:, : D * HW] = flat
    a = np.transpose(pad.reshape(SPC, NT, NROW), (0, 2, 1))  # [SPC, 125, 52]
    return {
        "b2": np.ascontiguousarray(b2),
        "a": np.ascontiguousarray(a),
        "temp": temp.astype(np.float32).reshape(1, 1),
    }


_IU, _JU = np.triu_indices(D)


def kernel(feat_map: np.ndarray, temperature: np.ndarray) -> np.ndarray:
    built = _build_kernel()
    nc = built["nc"]
    in_maps = [
        _prep_core_inputs(feat_map[c * SPC : (c + 1) * SPC], temperature)
        for c in range(NCORES)
    ]
    res = bass_utils.run_bass_kernel_spmd(
        nc, in_maps, core_ids=list(range(NCORES))
    )
    full = np.concatenate([res.results[c]["out"] for c in range(NCORES)], axis=0)
    return np.ascontiguousarray(full[:, _IU, _JU])


# revision 26
# speedup vs baseline: 4.8247x; 4.8247x over previous
"""Trainium2 Bass kernel for pairwise-channel-correlation pooling.

reference math (per sample, f: [256 ch, 25 pos]):
    G[i,j]  = sum_p (|f_ip + f_jp| - |f_ip - f_jp|)/2 * exp(T)
            = exp(T) * sum_p sign(f_ip) sign(f_jp) min(|f_ip|, |f_jp|)
    out     = G - rowmean(G) - colmean(G), then triu(row-major).

Device strategy (per core, 8 samples, pure data parallelism):
  - tiles of [(5 i-chans x 25 pos) = 125 partitions, 256 j] where
        t[q=(il,p), j] = clip(f_jp, -|f_ip|, +|f_ip|)
                       = sign(f_jp) * min(|f_ip|,|f_jp|)
    is ONE dual-op tensor_scalar (max, min) on the vector engine.
  - stationary +-1 matrices S_m = sign(a)*mask_m applied on the tensor
    engine (fp32r: 1 col/cycle for N=256, weights 0/+-1 so exact, fp32
    PSUM accumulation) reduce over p and scatter the 5 channel rows
    into two [128, 256] PSUM tiles.  Each S_m is dense [125, 128] but
    has only a 5-column nonzero window; the weight buffer's zeros are
    initialized once and per sample only the windows are rewritten via
    5 affine tensor_tensor ops (pattern * sign with step-0 APs).
  - ACT evacuates PSUM with the exp(T) scale fused and row-sums
    accumulated; column sums come from row sums by symmetry via two
    PE transposes; one scalar_tensor_tensor finishes the centering.
  - host does layout prep and the triu gather.
"""

import sys

sys.path.insert(0, "/opt/trn_rl_repo")

import numpy as np

import concourse.bass as bass
import concourse.tile as tile
from concourse import bacc, mybir
from concourse import bass_utils

F32 = mybir.dt.float32
F32R = mybir.dt.float32r

B, D, H, W = 64, 256, 5, 5
HW = H * W  # 25
NCORES = 8
SPC = B // NCORES  # samples per core
NROW = 125  # K rows per tile: 5 chans x 25 pos
NT = 52  # j-tiles per sample (51 full + 1 tail chan)

_BUILT = {}
_BUILT_LOOP = {}


def _mm_plan():
    """One entry per matmul: (g, psum_tile, rows=[(il, col)]).
    Tile g=25 straddles the two psum tiles and yields two entries."""
    plan = []
    for g in range(NT):
        i0 = 5 * g
        nch = 1 if g == NT - 1 else 5
        segs = {}
        for il in range(nch):
            i = i0 + il
            segs.setdefault(i // 128, []).append((il, i % 128))
        for pt, rows in sorted(segs.items()):
            plan.append((g, pt, rows))
    return plan


_PLAN = _mm_plan()
N_MM = len(_PLAN)  # 53
NA = N_MM  # a-table column m holds tile _PLAN[m][0]'s channel data
_G_SLOT = {}
for _m, (_g, _pt, _rows) in enumerate(_PLAN):
    _G_SLOT.setdefault(_g, _m)

SB_W = N_MM * 128  # S weight buffer free size (6784)


def _win_off(m):
    g, pt, rows = _PLAN[m]
    return 128 * m + rows[0][1]


def _build_pattern() -> np.ndarray:
    """[125, 5]: pattern[il*25:(il+1)*25, il] = 1."""
    p = np.zeros((NROW, 5), dtype=np.float32)
    for il in range(5):
        p[il * 25 : (il + 1) * 25, il] = 1.0
    return p


def _build_kernel(loop_k=None):
    cache = _BUILT if loop_k is None else _BUILT_LOOP
    if "nc" in cache and cache.get("loop_k") == loop_k:
        return cache
    Alu = mybir.AluOpType
    Act = mybir.ActivationFunctionType

    nc = bacc.Bacc("TRN2", target_bir_lowering=False, debug=False,
                   num_devices=NCORES)
    b2_d = nc.dram_tensor("b2", [SPC, NROW, D], F32, kind="ExternalInput")
    a_d = nc.dram_tensor("a", [SPC, NROW, NA], F32, kind="ExternalInput")
    temp_d = nc.dram_tensor("temp", [1, 1], F32, kind="ExternalInput")
    out_d = nc.dram_tensor("out", [SPC, D, D], F32, kind="ExternalOutput")
    pat_d = nc.inline_tensor(_build_pattern(), name="pattern")
    ident_d = nc.inline_tensor(np.eye(128, dtype=np.float32), name="ident")

    with tile.TileContext(nc) as tc:
        with (
            tc.tile_pool(name="const", bufs=1) as cpool,
            tc.tile_pool(name="inp", bufs=2) as ipool,
            tc.tile_pool(name="acol", bufs=2) as apool,
            tc.tile_pool(name="tg", bufs=4) as tpool,
            tc.tile_pool(name="post", bufs=3) as opool,
            tc.tile_pool(name="psum", bufs=1, space="PSUM") as pspool,
        ):
            # ---- constants ----
            pat = cpool.tile([NROW, 5], F32)
            nc.sync.dma_start(pat[:], pat_d.ap())
            ident = cpool.tile([128, 128], F32)
            nc.sync.dma_start(ident[:], ident_d.ap())
            tsb = cpool.tile([1, 1], F32)
            nc.sync.dma_start(tsb[:], temp_d.ap())
            expT = cpool.tile([1, 1], F32)
            nc.scalar.activation(expT[:], tsb[:], Act.Exp)
            expT_col = cpool.tile([128, 1], F32)
            nc.gpsimd.partition_broadcast(expT_col[:], expT[:])

            # persistent S weight buffers (zeros everywhere except the
            # per-sample 5-col windows), double-buffered by sample parity
            sbufs = [cpool.tile([NROW, SB_W], F32R, name=f"sw{i}")
                     for i in range(2)]
            for sb in sbufs:
                nc.vector.memset(sb[:].bitcast(F32), 0.0)

            import contextlib

            loop_cm = (
                tc.For_i(0, loop_k, 1) if loop_k is not None
                else contextlib.nullcontext()
            )
            with loop_cm:
                _emit_body(nc, tc, pat, ident, expT, expT_col, sbufs,
                           b2_d, a_d, out_d,
                           ipool, apool, tpool, opool, pspool)

    nc.compile()
    cache.clear()
    cache["nc"] = nc
    cache["loop_k"] = loop_k
    return cache


def _emit_sbuild(nc, sw, pat, signA):
    """Rewrite the nonzero windows of the persistent S buffer for this
    sample: S window of slot m = pattern_cols * sign(a col m)."""
    Alu = mybir.AluOpType

    def tt(out_ap, in0_ap, in1_ap):
        nc.vector.tensor_tensor(out_ap, in0_ap, in1_ap, Alu.mult)

    # group A: slots 0..24, window offset 133*m, width 5
    n = 25
    outA = sw[:, 0 : 133 * n].rearrange("p (n c) -> p n c", c=133)[:, :, 0:5]
    inA0 = pat[:].rearrange("p c -> p () c").broadcast_to([NROW, n, 5])
    inA1 = signA[:, 0:n].rearrange("p n -> p n ()").broadcast_to([NROW, n, 5])
    tt(outA, inA0, inA1)
    # group B: slots 27..51, window offset 133*m - 133, width 5
    offB = 133 * 27 - 133
    outB = sw[:, offB : offB + 133 * n].rearrange(
        "p (n c) -> p n c", c=133)[:, :, 0:5]
    inB1 = signA[:, 27:52].rearrange("p n -> p n ()").broadcast_to(
        [NROW, n, 5])
    tt(outB, inA0, inB1)
    # slot 25 (g25 part A): cols 125..127 <- il 0..2
    o25 = _win_off(25)
    tt(sw[:, o25 : o25 + 3], pat[:, 0:3],
       signA[:, 25:26].broadcast_to([NROW, 3]))
    # slot 26 (g25 part B): cols 0..1 <- il 3..4
    o26 = _win_off(26)
    tt(sw[:, o26 : o26 + 2], pat[:, 3:5],
       signA[:, 26:27].broadcast_to([NROW, 2]))
    # slot 52 (g51 tail): col 127 <- il 0
    o52 = _win_off(52)
    tt(sw[:, o52 : o52 + 1], pat[:, 0:1], signA[:, 52:53])


def _emit_body(nc, tc, pat, ident, expT, expT_col, sbufs,
               b2_d, a_d, out_d, ipool, apool, tpool, opool, pspool):
    Alu = mybir.AluOpType
    Act = mybir.ActivationFunctionType
    pt_last = {}
    for m, (g, pt, rows) in enumerate(_PLAN):
        pt_last[pt] = m

    for s in range(SPC):
        b2 = ipool.tile([NROW, D], F32)
        nc.sync.dma_start(b2[:], b2_d.ap()[s])
        a = ipool.tile([NROW, NA], F32)
        nc.sync.dma_start(a[:], a_d.ap()[s])

        absA = apool.tile([NROW, NA], F32)
        nc.scalar.activation(absA[:], a[:], Act.Abs)
        negA = apool.tile([NROW, NA], F32)
        nc.vector.tensor_scalar_mul(negA[:], absA[:], -1.0)
        signA = apool.tile([NROW, NA], F32)
        nc.scalar.sign(signA[:], a[:])

        sw = sbufs[s % 2]
        _emit_sbuild(nc, sw, pat, signA)

        psums = [
            pspool.tile([128, D], F32, tag="ps", name=f"ps_{s}_{t}", bufs=4)
            for t in range(2)
        ]

        tg = None
        cur_g = -1
        started = set()
        for m, (g, pt, rows) in enumerate(_PLAN):
            if g != cur_g:
                m0 = _G_SLOT[g]
                tg = tpool.tile([NROW, D], F32R, tag="tg",
                                name=f"tg_{s}_{g}")
                nc.vector.tensor_scalar(
                    tg[:], b2[:],
                    negA[:, m0 : m0 + 1], absA[:, m0 : m0 + 1],
                    Alu.max, Alu.min,
                )
                cur_g = g
            first = pt not in started
            started.add(pt)
            nc.tensor.matmul(
                psums[pt][:, :],
                sw[:, 128 * m : 128 * m + 128],
                tg[:],
                start=first, stop=(m == pt_last[pt]),
            )

        # psum -> sbuf with expT scale on ACT; fused row-sum accumulators
        gexp = [
            opool.tile([128, D], F32, tag="gexp", name=f"gexp_{s}_{t}", bufs=4)
            for t in range(2)
        ]
        rs = opool.tile([128, 2], F32, tag="rs")
        for t in range(2):
            nc.scalar.activation(
                gexp[t][:], psums[t][:, :], Act.Copy,
                scale=expT_col[:], accum_out=rs[:, t : t + 1],
            )

        # column sums via symmetry: transpose row-sum halves into one
        # contiguous [1, 256] psum row.
        ps_t = pspool.tile([1, D], F32, tag="pst", bufs=2)
        nc.tensor.transpose(ps_t[:, 0:128], rs[:, 0:1], ident[:])
        nc.tensor.transpose(ps_t[:, 128:256], rs[:, 1:2], ident[:])
        cs_s = opool.tile([1, D], F32, tag="cs")
        nc.vector.tensor_scalar_mul(cs_s[:], ps_t[:, :], 1.0 / D)
        cs_bc = opool.tile([128, D], F32, tag="csb")
        nc.gpsimd.partition_broadcast(cs_bc[:], cs_s[:])

        for t in range(2):
            rmean = opool.tile([128, 1], F32, tag="rm", name=f"rm_{s}_{t}")
            nc.vector.tensor_scalar_mul(rmean[:], rs[:, t : t + 1], 1.0 / D)
            o2 = opool.tile([128, D], F32, tag="o2")
            nc.vector.scalar_tensor_tensor(
                o2[:], gexp[t][:], rmean[:], cs_bc[:],
                Alu.subtract, Alu.subtract,
            )
            nc.sync.dma_start(out_d.ap()[s, 128 * t : 128 * (t + 1), :], o2[:])


def _prep_core_inputs(f_core: np.ndarray, temp: np.ndarray) -> dict:
    """f_core: [SPC, D, H, W] -> relayout for the device kernel."""
    fr = f_core.reshape(SPC, D, HW).astype(np.float32)
    # b2[s] = f^T tiled 5x along partitions: [(il,p), j] = f[j, p]
    ft = np.transpose(fr, (0, 2, 1))  # [SPC, 25, 256]
    b2 = np.tile(ft, (1, 5, 1))  # [SPC, 125, 256]
    # a-table: column m = channels of tile _PLAN[m][0] flattened (il,p),
    # tail tile zero-padded.
    flat = fr.reshape(SPC, D * HW)
    pad = np.zeros((SPC, NT * NROW), dtype=np.float32)
    pad[:, : D * HW] = flat
    acols = np.transpose(pad.reshape(SPC, NT, NROW), (0, 2, 1))  # [S,125,52]
    a = np.zeros((SPC, NROW, NA), dtype=np.float32)
    for m, (g, pt, rows) in enumerate(_PLAN):
        a[:, :, m] = acols[:, :, g]
    return {
        "b2": np.ascontiguousarray(b2),
        "a": np.ascontiguousarray(a),
        "temp": temp.astype(np.float32).reshape(1, 1),
    }


_IU, _JU = np.triu_indices(D)


def kernel(feat_map: np.ndarray, temperature: np.ndarray) -> np.ndarray:
    built = _build_kernel()
    nc = built["nc"]
    in_maps = [
        _prep_core_inputs(feat_map[c * SPC : (c + 1) * SPC], temperature)
        for c in range(NCORES)
    ]
    res = bass_utils.run_bass_kernel_spmd(
        nc, in_maps, core_ids=list(range(NCORES))
    )
    full = np.concatenate([res.results[c]["out"] for c in range(NCORES)], axis=0)
    return np.ascontiguousarray(full[:, _IU, _JU])
